# revision 1
# baseline (speedup 1.0000x reference)
"""Causal multi-head attention (B=4, T=2048, D=1024, 16 heads x 64) on 8 trn2 cores.

Sharding: tensor-parallel over heads, 2 heads per core. Every core receives the
full activations x (pre-transposed on host to [B, D, T], cast bf16) plus its 2
heads' worth of W_Q/W_K/W_V pre-arranged to [128, 8*128] bf16; it computes full
causal attention for its heads and writes out z^T plus the softmax denominator
row per head ([B, 2, 65, T] f32). The host normalizes and re-lays-out.

Device kernel layout choices (per core, HW ~266 us):
  - projections produce Q^T/K^T/V^T head-major [128(2h), T]; V^T is
    PE-transposed per 128-block into the AV stationary (v_aug).
  - scores computed transposed S^T[kt, qt] so the two heads run as concurrent
    PE row-tiles (K=64 at partition bases 0/64, ~4ns apart) into the two
    halves of one [128, 1024] PSUM pair; one Exp (scale=1/8 fused) per chunk.
  - causal mask applied post-exp with gpsimd affine_select (idle engine);
    diagonal chunks narrowed to their valid column range.
  - softmax denominator comes free from an all-ones column in v_aug (padded
    to 128 columns for fast weight load), accumulated in the same f32 PSUM
    as z^T; AV is software-pipelined one chunk behind the scores.
  - all matmuls bf16 with fp32 PSUM accumulation (end-to-end rel err ~5e-3);
    set PROJ_F32R=True for fp32r projections (~3.5e-3, ~13% slower).
"""

import os
import sys

for _p in ("/opt/trn_rl_repo", "/root/.axon_site/_ro/trn_rl_repo"):
    if os.path.isdir(_p) and _p not in sys.path:
        sys.path.insert(0, _p)

import ml_dtypes
import numpy as np

import concourse.bass as bass
import concourse.mybir as mybir
import concourse.tile as tile
from concourse import bacc
from concourse.bass import ds
from concourse.bass_utils import run_bass_kernel_spmd
from concourse.masks import make_identity

B, T, D = 4, 2048, 1024
NH, DH = 16, 64
NCORES = 8
HPC = NH // NCORES          # heads per core = 2
H2 = HPC * DH               # packed per-core head dim = 128
P = 128
QT = 512                    # query-tile width (psum bank limit for f32 out)
NQ4 = T // QT               # 4 query tiles
NCH = T // P                # 16 key chunks
KD = D // P                 # 8 contraction chunks
F32 = mybir.dt.float32
BF16 = mybir.dt.bfloat16
F32R = mybir.dt.float32r
PROJ_F32R = False  # fp32r projections: better precision, slightly slower
SCALE = 1.0 / np.sqrt(DH)   # 0.125


def _build(nc, tc, xT_d, w_d, cst_d, out_d):
    from contextlib import ExitStack

    AF = mybir.ActivationFunctionType
    OP = mybir.AluOpType
    MPB = QT // P  # 128-blocks per query tile = 4

    with ExitStack() as ctx:
        ep = ctx.enter_context
        const = ep(tc.tile_pool(name="const", bufs=1))
        xt_pool = ep(tc.tile_pool(name="xt", bufs=2 * KD + 1))
        qk_pool = ep(tc.tile_pool(name="qk", bufs=2))
        vt_pool = ep(tc.tile_pool(name="vt", bufs=3))
        vaug_pool = ep(tc.tile_pool(name="vaug", bufs=2))
        p_pool = ep(tc.tile_pool(name="pp", bufs=10))
        zt_pool = ep(tc.tile_pool(name="zt", bufs=2))
        ps_acc = ep(tc.tile_pool(name="ps_acc", bufs=2, space="PSUM"))
        ps_s = ep(tc.tile_pool(name="ps_s", bufs=2, space="PSUM"))
        ps_z = ep(tc.tile_pool(name="ps_z", bufs=2, space="PSUM"))

        # Startup: batch 0's x arrives as [128, 512] pieces in t4-major order
        # so the first projection group is never starved waiting on a full
        # 512 KB chunk; weights queue right after the very first piece.
        XDT = F32R if PROJ_F32R else BF16
        xts_pool = ep(tc.tile_pool(name="xts", bufs=NQ4 * KD))
        xch0p = [[None] * KD for _ in range(NQ4)]
        xch0p[0][0] = xts_pool.tile([P, QT], XDT, tag="xts", name="xts")
        nc.sync.dma_start(xch0p[0][0][:], xT_d[0, ds(0, P), ds(0, QT)])
        w_sb = {}
        for name in ("wq", "wk", "wv"):
            t = const.tile([P, KD, H2], XDT, tag=name)
            nc.sync.dma_start(t[:], w_d[name].rearrange("p (c h) -> p c h", c=KD))
            w_sb[name] = t
        for t4 in range(NQ4):
            for k in range(KD):
                if t4 == 0 and k == 0:
                    continue
                tt = xts_pool.tile([P, QT], XDT, tag="xts", name="xts")
                nc.sync.dma_start(tt[:], xT_d[0, ds(k * P, P), ds(t4 * QT, QT)])
                xch0p[t4][k] = tt

        ident = const.tile([P, P], BF16, tag="ident")
        make_identity(nc, ident)


        # v_aug double buffers: [kt, chunk, 64 v-cols | ones col | 63 zeros]
        # (padded to 128 columns so the AV weight load can use FWL)
        vaug = []
        for _bb in range(2):
            pair = []
            for h in range(HPC):
                v = vaug_pool.tile([P, NCH, P], BF16, tag=f"v{h}")
                nc.gpsimd.memset(v[:, :, DH:P], 0.0)
                nc.gpsimd.memset(v[:, :, DH : DH + 1], 1.0)
                pair.append(v)
            vaug.append(pair)

        for b in range(B):
            # ---- stream x^T chunks [128d, T] ----
            if b == 0:
                xch = None  # batch 0 uses the prefetched per-t4 pieces
            else:
                xch = []
                for k in range(KD):
                    xt_t = xt_pool.tile([P, T], XDT, tag="xt", name="xt_t")
                    nc.sync.dma_start(xt_t[:], xT_d[b, ds(k * P, P), :])
                    xch.append(xt_t)

            # ---- projections: Q^T, K^T (kept), V^T (transposed to v_aug) ----
            qt_sb = qk_pool.tile([P, T], BF16, tag="qt")
            kt_sb = qk_pool.tile([P, T], BF16, tag="kt")
            va = vaug[b % 2]
            for t4 in range(NQ4):
                for name, dst in (("wq", qt_sb), ("wk", kt_sb), ("wv", None)):
                    acc = ps_acc.tile([P, QT], F32, tag="acc")
                    for k in range(KD):
                        rhs = (
                            xch0p[t4][k][:]
                            if b == 0
                            else xch[k][:, ds(t4 * QT, QT)]
                        )
                        nc.tensor.matmul(
                            acc[:],
                            w_sb[name][:, k, :],
                            rhs,
                            start=(k == 0),
                            stop=(k == KD - 1),
                        )
                    if dst is not None:
                        nc.vector.tensor_copy(dst[:, ds(t4 * QT, QT)], acc[:])
                    else:
                        vt_t = vt_pool.tile([P, QT], BF16, tag="vt")
                        nc.vector.tensor_copy(vt_t[:], acc[:])
                        for m in range(MPB):
                            j = t4 * MPB + m
                            pt = ps_acc.tile([P, P], BF16, tag="acc", name="pt")
                            nc.tensor.transpose(
                                pt[:], vt_t[:, ds(m * P, P)], ident[:]
                            )
                            for h in range(HPC):
                                nc.vector.tensor_copy(
                                    va[h][:, j, 0:DH], pt[:, ds(h * DH, DH)]
                                )

            # ---- causal attention; scores for both heads side by side ----
            zt_sb = [
                zt_pool.tile([DH + 1, T], F32, tag=f"z{h}", name=f"ztb{h}")
                for h in range(HPC)
            ]
            for q4 in range(NQ4):
                njs = (q4 + 1) * MPB
                pz = [
                    ps_z.tile([P, QT], F32, tag="z", name="pz") for _ in range(HPC)
                ]
                pend = []  # (j, c0, exp tile) awaiting the AV matmuls
                for j in range(njs):
                    rdiag = j - q4 * MPB  # >=0 on diagonal-overlap chunks
                    last = j == njs - 1
                    c0 = 0 if rdiag < 0 else rdiag * P
                    w_hi = (rdiag + 1) * P if rdiag >= 0 else 0
                    nw = QT - c0
                    ss = ps_s.tile([P, 2 * QT], F32, tag="s")
                    pe = p_pool.tile([P, 2 * QT], BF16, tag="p", name="pe")
                    for h in range(HPC):
                        hp = ds(h * DH, DH)
                        nc.tensor.matmul(
                            ss[:, h * QT + c0 : (h + 1) * QT],
                            kt_sb[hp, ds(j * P, P)],
                            qt_sb[hp, ds(q4 * QT + c0, nw)],
                            start=True,
                            stop=True,
                        )
                    # one exp covering both heads' valid halves
                    if c0 == 0:
                        nc.scalar.activation(
                            pe[:, :], ss[:, :], AF.Exp, scale=float(SCALE)
                        )
                    else:
                        for h in range(HPC):
                            nc.scalar.activation(
                                pe[:, h * QT + c0 : (h + 1) * QT],
                                ss[:, h * QT + c0 : (h + 1) * QT],
                                AF.Exp,
                                scale=float(SCALE),
                            )
                    if rdiag >= 0:
                        # keep iff qt >= kt  <=>  (col - p - 128*rdiag) >= 0
                        for h in range(HPC):
                            nc.gpsimd.affine_select(
                                out=pe[:, h * QT + c0 : h * QT + w_hi],
                                in_=pe[:, h * QT + c0 : h * QT + w_hi],
                                compare_op=OP.is_ge,
                                fill=0.0,
                                base=c0 - rdiag * P,
                                pattern=[[1, w_hi - c0]],
                                channel_multiplier=-1,
                            )
                    pend.append((j, c0, pe))
                    # software-pipeline: AV runs one chunk behind the scores
                    if len(pend) > 3 or last:
                        for jj, cc0, ppe in pend if last else [pend[0]]:
                            for h in range(HPC):
                                nc.tensor.matmul(
                                    pz[h][:, cc0:QT],
                                    va[h][:, jj, :],
                                    ppe[:, h * QT + cc0 : (h + 1) * QT],
                                    start=(jj == 0),
                                    stop=(jj == njs - 1),
                                    skip_group_check=True,
                                )
                        pend = [] if last else pend[1:]

                for h in range(HPC):
                    nc.vector.tensor_copy(
                        zt_sb[h][:, ds(q4 * QT, QT)], pz[h][0 : DH + 1, :]
                    )
                    # z^T (+ denominator row) raw; host divides and transposes
                    nc.sync.dma_start(
                        out_d[b, h, :, ds(q4 * QT, QT)],
                        zt_sb[h][:, ds(q4 * QT, QT)],
                    )


def build_bass():
    nc = bacc.Bacc(None, target_bir_lowering=False)
    xT_d = nc.declare_dram_parameter(
        "xT", [B, D, T], F32R if PROJ_F32R else BF16, isOutput=False
    )
    w_d = {
        name: nc.declare_dram_parameter(
            name, [P, KD * H2], F32R if PROJ_F32R else BF16, isOutput=False
        )
        for name in ("wq", "wk", "wv")
    }
    cst_d = {}
    out_d = nc.declare_dram_parameter(
        "out", [B, HPC, DH + 1, T], F32, isOutput=True
    )
    with tile.TileContext(nc) as tc:
        _build(nc, tc, xT_d, w_d, cst_d, out_d)
    nc.compile()
    return nc


_CACHE = {}


def _get_nc():
    if "nc" not in _CACHE:
        _CACHE["nc"] = build_bass()
    return _CACHE["nc"]


def make_in_maps(x, W_K, W_Q, W_V):
    x = np.asarray(x, dtype=np.float32)
    xT = np.ascontiguousarray(np.transpose(x, (0, 2, 1)))
    if not PROJ_F32R:
        xT = xT.astype(ml_dtypes.bfloat16)
    in_maps = []
    for c in range(NCORES):
        sl = slice(c * HPC, (c + 1) * HPC)

        def wt(w):
            w = np.asarray(w, dtype=np.float32)
            wt_ = w[sl].reshape(H2, D).T  # [D, H2]
            wt_ = wt_.reshape(KD, P, H2).transpose(1, 0, 2).reshape(P, KD * H2)
            wt_ = np.ascontiguousarray(wt_)
            return wt_ if PROJ_F32R else wt_.astype(ml_dtypes.bfloat16)

        in_maps.append({"xT": xT, "wq": wt(W_Q), "wk": wt(W_K), "wv": wt(W_V)})
    return in_maps


def kernel(x, W_K, W_Q, W_V, _trace=False, _trace_kwargs=None):
    in_maps = make_in_maps(x, W_K, W_Q, W_V)
    res = run_bass_kernel_spmd(
        _get_nc(),
        in_maps,
        list(range(NCORES)),
        trace=_trace,
        **(_trace_kwargs or {}),
    )
    _CACHE["last_results"] = res
    outs = []
    for c in range(NCORES):
        zt = np.asarray(res.results[c]["out"])  # [B, HPC, DH+1, T]
        z = zt[:, :, :DH, :] / zt[:, :, DH : DH + 1, :]
        outs.append(np.transpose(z, (0, 3, 1, 2)).reshape(B, T, H2))
    return np.concatenate(outs, axis=2)



# revision 3
# speedup vs baseline: 1.0115x; 1.0115x over previous
"""Causal multi-head attention (B=4, T=2048, D=1024, 16 heads x 64) on 8 trn2 cores.

Sharding: batch x heads hybrid. Core c owns batch c//2 and heads
(c%2)*8 .. +8 (four packed head-pairs of 128). Each core receives x^T for its
batch ([D, T] bf16) plus its 8 heads' W_Q/W_K (stationary layout) and W_V^T
(moving layout); it computes full causal attention for its heads and writes
z^T plus the softmax denominator row per head ([PAIRS, 2, 65, T] f32). The
host normalizes and re-lays-out.

Device kernel design (per core):
  - Q^T/K^T projections as 8-matmul accumulation units per (pair, 512-col
    quarter); stationary weights, moving x (N=512, full PE rate).
  - V produced directly in [t, h] orientation: stationary x^T[d,t] chunk,
    moving W_V^T (N=512) -> no PE transposes at all; DVE copies slice the
    PSUM result into per-(pair,head) v_aug tiles ([128 kt, 16 chunk, 64 v |
    ones | pad]) whose ones-column yields the softmax denominator for free.
  - scores computed transposed S^T[kt, qt]; the two heads of a pair run as
    concurrent PE row-tiles (K=64 at partition bases 0/64) into the two
    halves of one [128, 1024] f32 PSUM; ONE exp (scale=1/8 fused) per chunk
    covers both heads (diagonal chunks span the dead middle columns; that
    garbage is never read downstream).
  - causal mask applied post-exp with gpsimd affine_select (idle engine).
  - AV is software-pipelined a few chunks behind the scores and accumulates
    z^T + denominator in f32 PSUM.
  - THE SCHEDULING POINT: exp on ScalarE (~166us/core) and matmuls on PE
    (~169us/core) are nearly balanced, and engine queues are FIFO - so the
    emission order interleaves projection/V units between attention chunks
    (ratio-paced + deadline-forced) across ALL four pairs' attention,
    keeping both engines busy instead of serializing phase by phase.
"""

import os
import sys
from collections import deque

for _p in ("/opt/trn_rl_repo", "/root/.axon_site/_ro/trn_rl_repo"):
    if os.path.isdir(_p) and _p not in sys.path:
        sys.path.insert(0, _p)

import ml_dtypes
import numpy as np

import concourse.bass as bass
import concourse.mybir as mybir
import concourse.tile as tile
from concourse import bacc
from concourse.bass import ds
from concourse.bass_utils import run_bass_kernel_spmd

B, T, D = 4, 2048, 1024
NH, DH = 16, 64
NCORES = 8
HPB = 8                     # heads per core (batch x head sharding)
PAIRS = HPB // 2            # packed head-pairs per core = 4
H2 = 2 * DH                 # packed pair dim = 128
P = 128
QT = 512                    # query-tile width (psum bank limit for f32 out)
NQ4 = T // QT               # 4 query tiles
NCH = T // P                # 16 key chunks
KD = D // P                 # 8 contraction chunks
F32 = mybir.dt.float32
BF16 = mybir.dt.bfloat16
SCALE = 1.0 / np.sqrt(DH)   # 0.125


def _build(nc, tc, xT_d, w_d, out_d):
    from contextlib import ExitStack

    AF = mybir.ActivationFunctionType
    OP = mybir.AluOpType

    with ExitStack() as ctx:
        ep = ctx.enter_context
        const = ep(tc.tile_pool(name="const", bufs=1))
        xt_pool = ep(tc.tile_pool(name="xt", bufs=1))
        qk_pool = ep(tc.tile_pool(name="qk", bufs=1))
        va_pool = ep(tc.tile_pool(name="va", bufs=1))
        p_pool = ep(tc.tile_pool(name="pp", bufs=8))
        zt_pool = ep(tc.tile_pool(name="zt", bufs=4))
        ps_acc = ep(tc.tile_pool(name="ps_acc", bufs=2, space="PSUM"))
        ps_s = ep(tc.tile_pool(name="ps_s", bufs=2, space="PSUM"))
        ps_z = ep(tc.tile_pool(name="ps_z", bufs=2, space="PSUM"))

        # ---- weights first (small), then x^T quarters t4-major so the
        # first projection units are never starved ----
        w_sb = {}
        for name in ("wq", "wk"):
            t = const.tile([P, KD, PAIRS, H2], BF16, tag=name, name=name)
            nc.sync.dma_start(
                t[:], w_d[name].rearrange("p (c r h) -> p c r h", c=KD, r=PAIRS)
            )
            w_sb[name] = t
        wv_sb = const.tile([P, KD, HPB * DH], BF16, tag="wv", name="wv_sb")
        nc.sync.dma_start(wv_sb[:], w_d["wv"].rearrange("p (c h) -> p c h", c=KD))

        xt = [
            xt_pool.tile([P, T], BF16, tag=f"x{k}", name="xt") for k in range(KD)
        ]
        for t4 in range(NQ4):
            for k in range(KD):
                nc.sync.dma_start(
                    xt[k][:, ds(t4 * QT, QT)], xT_d[ds(k * P, P), ds(t4 * QT, QT)]
                )

        qt = [
            qk_pool.tile([P, T], BF16, tag=f"qt{p}", name="qt") for p in range(PAIRS)
        ]
        kt = [
            qk_pool.tile([P, T], BF16, tag=f"kt{p}", name="kt") for p in range(PAIRS)
        ]
        # v_aug: [kt, chunk, 64 v-cols | ones col | 63 zeros]
        va = [
            [
                va_pool.tile([P, NCH, P], BF16, tag=f"v{p}{a}", name="va")
                for a in range(2)
            ]
            for p in range(PAIRS)
        ]
        for p in range(PAIRS):
            for a in range(2):
                nc.gpsimd.memset(va[p][a][:, :, DH:P], 0.0)
                nc.gpsimd.memset(va[p][a][:, :, DH : DH + 1], 1.0)

        # ---- background work units (projections + V), emitted between
        # attention chunks by the driver below ----
        def qk_unit(name, p, t4):
            dst = qt[p] if name == "wq" else kt[p]

            def emit():
                acc = ps_acc.tile([P, QT], F32, tag="acc", name="acc")
                for k in range(KD):
                    nc.tensor.matmul(
                        acc[:],
                        w_sb[name][:, k, p, :],
                        xt[k][:, ds(t4 * QT, QT)],
                        start=(k == 0),
                        stop=(k == KD - 1),
                    )
                nc.vector.tensor_copy(dst[:, ds(t4 * QT, QT)], acc[:])

            return emit

        def v_unit(j):
            def emit():
                pv = ps_acc.tile([P, HPB * DH], F32, tag="acc", name="pv")
                for k in range(KD):
                    nc.tensor.matmul(
                        pv[:],
                        xt[k][:, ds(j * P, P)],
                        wv_sb[:, k, :],
                        start=(k == 0),
                        stop=(k == KD - 1),
                    )
                for p in range(PAIRS):
                    for a in range(2):
                        nc.vector.tensor_copy(
                            va[p][a][:, j, 0:DH], pv[:, ds((2 * p + a) * DH, DH)]
                        )

            return emit

        # deadline bookkeeping: qk_done[p] = quarters projected; v_done = chunks
        state = {"v_done": 0, "qk_done": [[0, 0] for _ in range(PAIRS)]}

        def emit_qk(p, upto_t4):
            for t4 in range(state["qk_done"][p][0], upto_t4):
                qk_unit("wq", p, t4)()
                state["qk_done"][p][0] = t4 + 1
            for t4 in range(state["qk_done"][p][1], upto_t4):
                qk_unit("wk", p, t4)()
                state["qk_done"][p][1] = t4 + 1

        def emit_v(upto_j):
            while state["v_done"] < upto_j:
                v_unit(state["v_done"])()
                state["v_done"] += 1

        # bg queue in deadline order; each entry = (kind, arg) popped by ratio
        bg = deque()
        for q4 in range(NQ4):
            for p in range(PAIRS):
                if not (q4 == 0 and p == 0):
                    bg.append(("qk", p, q4 + 1))
            bg.append(("v", 4 * (q4 + 1)))

        def pop_bg():
            if not bg:
                return False
            item = bg.popleft()
            if item[0] == "qk":
                _, p, upto = item
                if state["qk_done"][p][0] >= upto and state["qk_done"][p][1] >= upto:
                    return pop_bg()
                # emit one quarter step (q then k) toward the deadline
                t4q = state["qk_done"][p][0]
                if t4q < upto:
                    qk_unit("wq", p, t4q)()
                    state["qk_done"][p][0] = t4q + 1
                t4k = state["qk_done"][p][1]
                if t4k < upto:
                    qk_unit("wk", p, t4k)()
                    state["qk_done"][p][1] = t4k + 1
                if state["qk_done"][p][0] < upto or state["qk_done"][p][1] < upto:
                    bg.appendleft(item)
            else:
                _, upto = item
                if state["v_done"] >= upto:
                    return pop_bg()
                v_unit(state["v_done"])()
                state["v_done"] += 1
                if state["v_done"] < upto:
                    bg.appendleft(item)
            return True

        # total bg emission units for ratio pacing: 32 qk quarters*2 + 16 v
        # measured in "unit steps" as popped above (qk pops a q+k pair)
        total_bg_steps = 32 + 16  # 32 qk pair-steps + 16 v chunks
        total_chunks = PAIRS * sum(4 * (q4 + 1) for q4 in range(NQ4))  # 160
        done_bg = [0]
        done_ch = [0]

        def inject_bg():
            # keep emitted bg proportional to attention progress
            while (bg and
                   done_bg[0] * total_chunks < total_bg_steps * done_ch[0]):
                if not pop_bg():
                    break
                done_bg[0] += 1

        # ---- prologue: just enough projection for the first q4 tile ----
        emit_qk(0, 1)
        state_done0 = state["qk_done"][0]
        assert state_done0 == [1, 1]

        # ---- attention: q4 tiles round-robin across pairs ----
        zrow = DH + 1
        for q4 in range(NQ4):
            njs = (q4 + 1) * (QT // P)
            for p in range(PAIRS):
                # deadlines for this tile
                emit_qk(p, q4 + 1)
                emit_v(njs)
                pz = [
                    ps_z.tile([P, QT], F32, tag="z", name="pz") for _ in range(2)
                ]
                pend = []  # (j, c0, exp tile) awaiting the AV matmuls
                for j in range(njs):
                    rdiag = j - q4 * (QT // P)
                    last = j == njs - 1
                    c0 = 0 if rdiag < 0 else rdiag * P
                    w_hi = (rdiag + 1) * P if rdiag >= 0 else 0
                    nw = QT - c0
                    ss = ps_s.tile([P, 2 * QT], F32, tag="s", name="ss")
                    pe = p_pool.tile([P, 2 * QT], BF16, tag="p", name="pe")
                    for a in range(2):
                        hp = ds(a * DH, DH)
                        nc.tensor.matmul(
                            ss[:, a * QT + c0 : (a + 1) * QT],
                            kt[p][hp, ds(j * P, P)],
                            qt[p][hp, ds(q4 * QT + c0, nw)],
                            start=True,
                            stop=True,
                        )
                    # one exp covering both heads' valid column blocks via a
                    # strided AP (no read of the dead middle columns)
                    nc.scalar.activation(
                        pe.rearrange("p (a q) -> p a q", a=2)[:, :, c0:QT],
                        ss.rearrange("p (a q) -> p a q", a=2)[:, :, c0:QT],
                        AF.Exp,
                        scale=float(SCALE),
                    )
                    if rdiag >= 0:
                        # keep iff qt >= kt  <=>  (col - p - 128*rdiag) >= 0
                        for a in range(2):
                            nc.gpsimd.affine_select(
                                out=pe[:, a * QT + c0 : a * QT + w_hi],
                                in_=pe[:, a * QT + c0 : a * QT + w_hi],
                                compare_op=OP.is_ge,
                                fill=0.0,
                                base=c0 - rdiag * P,
                                pattern=[[1, w_hi - c0]],
                                channel_multiplier=-1,
                            )
                    pend.append((j, c0, pe))
                    # software-pipeline: AV runs a few chunks behind scores
                    if len(pend) > 3 or last:
                        for jj, cc0, ppe in pend if last else [pend[0]]:
                            for a in range(2):
                                nc.tensor.matmul(
                                    pz[a][:, cc0:QT],
                                    va[p][a][:, jj, :],
                                    ppe[:, a * QT + cc0 : (a + 1) * QT],
                                    start=(jj == 0),
                                    stop=(jj == njs - 1),
                                    skip_group_check=True,
                                )
                        pend = [] if last else pend[1:]
                    done_ch[0] += 1
                    inject_bg()

                for a in range(2):
                    zt_t = zt_pool.tile([zrow, QT], F32, tag="zt", name="zt_t")
                    nc.vector.tensor_copy(zt_t[:], pz[a][0:zrow, :])
                    nc.sync.dma_start(
                        out_d[p, a, :, ds(q4 * QT, QT)], zt_t[:]
                    )

        # drain any remaining bg (shouldn't happen)
        while bg:
            if not pop_bg():
                break


def build_bass():
    nc = bacc.Bacc(None, target_bir_lowering=False)
    xT_d = nc.declare_dram_parameter("xT", [D, T], BF16, isOutput=False)
    w_d = {
        "wq": nc.declare_dram_parameter(
            "wq", [P, KD * PAIRS * H2], BF16, isOutput=False
        ),
        "wk": nc.declare_dram_parameter(
            "wk", [P, KD * PAIRS * H2], BF16, isOutput=False
        ),
        "wv": nc.declare_dram_parameter(
            "wv", [P, KD * HPB * DH], BF16, isOutput=False
        ),
    }
    out_d = nc.declare_dram_parameter(
        "out", [PAIRS, 2, DH + 1, T], F32, isOutput=True
    )
    with tile.TileContext(nc) as tc:
        _build(nc, tc, xT_d, w_d, out_d)
    nc.compile()
    return nc


_CACHE = {}


def _get_nc():
    if "nc" not in _CACHE:
        _CACHE["nc"] = build_bass()
    return _CACHE["nc"]


def make_in_maps(x, W_K, W_Q, W_V):
    x = np.asarray(x, dtype=np.float32)
    in_maps = []
    for c in range(NCORES):
        b = c // 2
        hb = (c % 2) * HPB
        xT = np.ascontiguousarray(x[b].T).astype(ml_dtypes.bfloat16)

        def stat(w):  # stationary layout for Q/K: [P, KD, PAIRS, H2]
            w = np.asarray(w, dtype=np.float32)
            arr = np.empty((P, KD, PAIRS, H2), np.float32)
            for p in range(PAIRS):
                wp = w[hb + 2 * p : hb + 2 * p + 2].reshape(H2, D).T  # [D, H2]
                arr[:, :, p, :] = wp.reshape(KD, P, H2).transpose(1, 0, 2)
            return np.ascontiguousarray(
                arr.reshape(P, KD * PAIRS * H2)
            ).astype(ml_dtypes.bfloat16)

        def mov(w):  # moving layout for V: [P, KD, HPB*DH]
            w = np.asarray(w, dtype=np.float32)
            wt = w[hb : hb + HPB].reshape(HPB * DH, D).T  # [D, 8*64] head-major
            wt = wt.reshape(KD, P, HPB * DH).transpose(1, 0, 2)
            return np.ascontiguousarray(
                wt.reshape(P, KD * HPB * DH)
            ).astype(ml_dtypes.bfloat16)

        in_maps.append(
            {"xT": xT, "wq": stat(W_Q), "wk": stat(W_K), "wv": mov(W_V)}
        )
    return in_maps


def kernel(x, W_K, W_Q, W_V, _trace=False, _trace_kwargs=None):
    in_maps = make_in_maps(x, W_K, W_Q, W_V)
    res = run_bass_kernel_spmd(
        _get_nc(),
        in_maps,
        list(range(NCORES)),
        trace=_trace,
        **(_trace_kwargs or {}),
    )
    _CACHE["last_results"] = res
    out = np.empty((B, T, NH * DH), np.float32)
    for c in range(NCORES):
        zt = np.asarray(res.results[c]["out"])  # [PAIRS, 2, DH+1, T]
        z = zt[:, :, :DH, :] / zt[:, :, DH : DH + 1, :]
        b = c // 2
        hb = (c % 2) * HPB
        for p in range(PAIRS):
            for a in range(2):
                h = hb + 2 * p + a
                out[b, :, h * DH : (h + 1) * DH] = z[p, a].T
    return out


# revision 11
# speedup vs baseline: 1.0811x; 1.0688x over previous
"""Causal multi-head attention (B=4, T=2048, D=1024, 16 heads x 64) on 8 trn2 cores.

Sharding: batch x heads hybrid. Core c owns batch c//2 and heads
(c%2)*8 .. +8 (four packed head-pairs of 128). Each core receives x^T for its
batch ([D, T] bf16) plus its 8 heads' W_Q/W_K (stationary layout) and W_V^T
(moving layout); it computes full causal attention for its heads and writes
z^T plus the softmax denominator row per head ([PAIRS, 2, 65, T] f32). The
host normalizes and re-lays-out.

Device kernel design (per core):
  - Q^T/K^T projections as 8-matmul accumulation units per (pair, 512-col
    quarter); stationary weights, moving x (N=512, full PE rate).
  - V produced directly in [t, h] orientation: stationary x^T[d,t] chunk,
    moving W_V^T (N=512) -> no PE transposes at all; DVE copies slice the
    PSUM result into per-(pair,head) v_aug tiles ([128 kt, 16 chunk, 64 v |
    ones | pad]) whose ones-column yields the softmax denominator for free.
  - scores computed transposed S^T[kt, qt]; the two heads of a pair run as
    concurrent PE row-tiles (K=64 at partition bases 0/64) into the two
    halves of one [128, 1024] f32 PSUM; ONE exp (scale=1/8 fused) per chunk
    covers both heads (diagonal chunks span the dead middle columns; that
    garbage is never read downstream).
  - causal mask applied post-exp with gpsimd affine_select (idle engine).
  - AV is software-pipelined a few chunks behind the scores and accumulates
    z^T + denominator in f32 PSUM.
  - THE SCHEDULING POINT: exp on ScalarE (~166us/core) and matmuls on PE
    (~169us/core) are nearly balanced, and engine queues are FIFO - so the
    emission order interleaves projection/V units between attention chunks
    (ratio-paced + deadline-forced) across ALL four pairs' attention,
    keeping both engines busy instead of serializing phase by phase.
"""

import os
import sys

for _p in ("/opt/trn_rl_repo", "/root/.axon_site/_ro/trn_rl_repo"):
    if os.path.isdir(_p) and _p not in sys.path:
        sys.path.insert(0, _p)

import ml_dtypes
import numpy as np

import concourse.bass as bass
import concourse.mybir as mybir
import concourse.tile as tile
from concourse import bacc
from concourse.bass import ds
from concourse.bass_utils import run_bass_kernel_spmd

B, T, D = 4, 2048, 1024
NH, DH = 16, 64
NCORES = 8
HPB = 8                     # heads per core (batch x head sharding)
PAIRS = HPB // 2            # packed head-pairs per core = 4
H2 = 2 * DH                 # packed pair dim = 128
P = 128
QT = 512                    # query-tile width (psum bank limit for f32 out)
NQ4 = T // QT               # 4 query tiles
NCH = T // P                # 16 key chunks
KD = D // P                 # 8 contraction chunks
F32 = mybir.dt.float32
BF16 = mybir.dt.bfloat16
SCALE = 1.0 / np.sqrt(DH)   # 0.125


def _build(nc, tc, xT_d, w_d, out_d):
    from contextlib import ExitStack

    AF = mybir.ActivationFunctionType
    OP = mybir.AluOpType

    with ExitStack() as ctx:
        ep = ctx.enter_context
        const = ep(tc.tile_pool(name="const", bufs=1))
        xt_pool = ep(tc.tile_pool(name="xt", bufs=1))
        qk_pool = ep(tc.tile_pool(name="qk", bufs=1))
        va_pool = ep(tc.tile_pool(name="va", bufs=1))
        p_pool = ep(tc.tile_pool(name="pp", bufs=8))
        zt_pool = ep(tc.tile_pool(name="zt", bufs=4))
        ps_acc = ep(tc.tile_pool(name="ps_acc", bufs=2, space="PSUM"))
        ps_s = ep(tc.tile_pool(name="ps_s", bufs=2, space="PSUM"))
        ps_z = ep(tc.tile_pool(name="ps_z", bufs=2, space="PSUM"))

        # ---- DMA order matters: pair-0 Q/K weights, then the first x^T
        # quarter, so the first projection unit can start ~1.5us in ----
        w_sb = {}
        for name in ("wq", "wk"):
            w_sb[name] = const.tile(
                [P, PAIRS, KD, H2], BF16, tag=name, name=name
            )
        wv_sb = const.tile([P, KD, HPB * DH], BF16, tag="wv", name="wv_sb")
        xt = [
            xt_pool.tile([P, T], BF16, tag=f"x{k}", name="xt") for k in range(KD)
        ]

        def dma_w_pair(p):
            for name in ("wq", "wk"):
                nc.sync.dma_start(
                    w_sb[name][:, p],
                    w_d[name].rearrange(
                        "p (r c h) -> p r c h", r=PAIRS, c=KD
                    )[:, p],
                )

        dma_w_pair(0)
        for k in range(KD):
            nc.sync.dma_start(
                xt[k][:, ds(0, QT)], xT_d[ds(k * P, P), ds(0, QT)]
            )
        nc.sync.dma_start(wv_sb[:], w_d["wv"].rearrange("p (c h) -> p c h", c=KD))
        dma_w_pair(1)
        for t4 in range(1, NQ4):
            for k in range(KD):
                nc.sync.dma_start(
                    xt[k][:, ds(t4 * QT, QT)], xT_d[ds(k * P, P), ds(t4 * QT, QT)]
                )
            if t4 + 1 < PAIRS:
                dma_w_pair(t4 + 1)

        qt = [
            qk_pool.tile([P, T], BF16, tag=f"qt{p}", name="qt") for p in range(PAIRS)
        ]
        kt = [
            qk_pool.tile([P, T], BF16, tag=f"kt{p}", name="kt") for p in range(PAIRS)
        ]
        # v_aug: [kt, chunk, 64 v-cols | ones col | 63 zeros]
        va = [
            [
                va_pool.tile([P, NCH, P], BF16, tag=f"v{p}{a}", name="va")
                for a in range(2)
            ]
            for p in range(PAIRS)
        ]
        for p in range(PAIRS):
            for a in range(2):
                nc.gpsimd.memset(va[p][a][:, :, DH:P], 0.0)
                nc.gpsimd.memset(va[p][a][:, :, DH : DH + 1], 1.0)

        # ---- background work units (projections + V), emitted between
        # attention chunks by the driver below ----
        def qk_unit(name, p, t4):
            dst = qt[p] if name == "wq" else kt[p]

            def emit():
                acc = ps_acc.tile([P, QT], F32, tag="acc", name="acc")
                for k in range(KD):
                    nc.tensor.matmul(
                        acc[:],
                        w_sb[name][:, p, k, :],
                        xt[k][:, ds(t4 * QT, QT)],
                        start=(k == 0),
                        stop=(k == KD - 1),
                    )
                nc.vector.tensor_copy(dst[:, ds(t4 * QT, QT)], acc[:])

            return emit

        def v_unit(j):
            def emit():
                pv = ps_acc.tile([P, HPB * DH], F32, tag="acc", name="pv")
                for k in range(KD):
                    nc.tensor.matmul(
                        pv[:],
                        xt[k][:, ds(j * P, P)],
                        wv_sb[:, k, :],
                        start=(k == 0),
                        stop=(k == KD - 1),
                    )
                for p in range(PAIRS):
                    for a in range(2):
                        nc.vector.tensor_copy(
                            va[p][a][:, j, 0:DH], pv[:, ds((2 * p + a) * DH, DH)]
                        )

            return emit

        # ---- EDF schedule of projection/V units between attention chunks.
        # Tiles run q4-major, pair-minor; each unit gets a chunk-indexed
        # deadline (the chunk before which it must be emitted), and a rate
        # pacer walks the deadline-sorted list early so the PE never takes
        # a multi-unit burst that starves the exp stream. ----
        MPB = QT // P
        tile_start = {}
        cs = 0
        for q4 in range(NQ4):
            for p in range(PAIRS):
                tile_start[(q4, p)] = cs
                cs += (q4 + 1) * MPB
        total_chunks = cs  # 160

        units = []  # (deadline, order, emit_fn)
        for p in range(PAIRS):
            for t4 in range(NQ4):
                dl = tile_start[(t4, p)]
                units.append((dl, 0, qk_unit("wq", p, t4)))
                units.append((dl, 1, qk_unit("wk", p, t4)))
        for j in range(NCH):
            q4 = j // MPB
            dl = tile_start[(q4, 0)] + (j - q4 * MPB)
            units.append((dl, 2, v_unit(j)))
        units.sort(key=lambda u: (u[0], u[1]))
        n_units = len(units)
        uidx = [0]

        def inject_bg(c):
            # deadline-forced, then rate-paced (n_units spread over chunks)
            while uidx[0] < n_units and units[uidx[0]][0] <= c:
                units[uidx[0]][2]()
                uidx[0] += 1
            while (uidx[0] < n_units
                   and uidx[0] * total_chunks < n_units * c):
                units[uidx[0]][2]()
                uidx[0] += 1

        # ---- attention: q4 tiles round-robin across pairs ----
        zrow = DH + 1
        gchunk = [0]
        for q4 in range(NQ4):
            njs = (q4 + 1) * MPB
            for p in range(PAIRS):
                pz = [
                    ps_z.tile([P, QT], F32, tag="z", name="pz") for _ in range(2)
                ]
                pend = []  # (j, c0, exp tile) awaiting the AV matmuls
                for j in range(njs):
                    inject_bg(gchunk[0])
                    gchunk[0] += 1
                    rdiag = j - q4 * (QT // P)
                    last = j == njs - 1
                    c0 = 0 if rdiag < 0 else rdiag * P
                    w_hi = (rdiag + 1) * P if rdiag >= 0 else 0
                    nw = QT - c0
                    ss = ps_s.tile([P, 2 * QT], F32, tag="s", name="ss")
                    pe = p_pool.tile([P, 2 * QT], BF16, tag="p", name="pe")
                    for a in range(2):
                        hp = ds(a * DH, DH)
                        nc.tensor.matmul(
                            ss[:, a * QT + c0 : (a + 1) * QT],
                            kt[p][hp, ds(j * P, P)],
                            qt[p][hp, ds(q4 * QT + c0, nw)],
                            start=True,
                            stop=True,
                        )
                    # one exp covering both heads' valid column blocks via a
                    # strided AP (no read of the dead middle columns)
                    nc.scalar.activation(
                        pe.rearrange("p (a q) -> p a q", a=2)[:, :, c0:QT],
                        ss.rearrange("p (a q) -> p a q", a=2)[:, :, c0:QT],
                        AF.Exp,
                        scale=float(SCALE),
                    )
                    if rdiag >= 0:
                        # keep iff qt >= kt  <=>  (col - p - 128*rdiag) >= 0
                        for a in range(2):
                            nc.gpsimd.affine_select(
                                out=pe[:, a * QT + c0 : a * QT + w_hi],
                                in_=pe[:, a * QT + c0 : a * QT + w_hi],
                                compare_op=OP.is_ge,
                                fill=0.0,
                                base=c0 - rdiag * P,
                                pattern=[[1, w_hi - c0]],
                                channel_multiplier=-1,
                            )
                    pend.append((j, c0, pe))
                    # software-pipeline: AV runs a few chunks behind scores
                    if len(pend) > 3 or last:
                        for jj, cc0, ppe in pend if last else [pend[0]]:
                            for a in range(2):
                                nc.tensor.matmul(
                                    pz[a][:, cc0:QT],
                                    va[p][a][:, jj, :],
                                    ppe[:, a * QT + cc0 : (a + 1) * QT],
                                    start=(jj == 0),
                                    stop=(jj == njs - 1),
                                    skip_group_check=True,
                                )
                        pend = [] if last else pend[1:]

                for a in range(2):
                    zt_t = zt_pool.tile([zrow, QT], F32, tag="zt", name="zt_t")
                    nc.vector.tensor_copy(zt_t[:], pz[a][0:zrow, :])
                    nc.sync.dma_start(
                        out_d[p, a, :, ds(q4 * QT, QT)], zt_t[:]
                    )

        # drain any remaining units (shouldn't happen)
        while uidx[0] < n_units:
            units[uidx[0]][2]()
            uidx[0] += 1


def build_bass():
    nc = bacc.Bacc(None, target_bir_lowering=False)
    xT_d = nc.declare_dram_parameter("xT", [D, T], BF16, isOutput=False)
    w_d = {
        "wq": nc.declare_dram_parameter(
            "wq", [P, KD * PAIRS * H2], BF16, isOutput=False
        ),
        "wk": nc.declare_dram_parameter(
            "wk", [P, KD * PAIRS * H2], BF16, isOutput=False
        ),
        "wv": nc.declare_dram_parameter(
            "wv", [P, KD * HPB * DH], BF16, isOutput=False
        ),
    }
    out_d = nc.declare_dram_parameter(
        "out", [PAIRS, 2, DH + 1, T], F32, isOutput=True
    )
    with tile.TileContext(nc) as tc:
        _build(nc, tc, xT_d, w_d, out_d)
    nc.compile()
    return nc


_CACHE = {}


def _get_nc():
    if "nc" not in _CACHE:
        _CACHE["nc"] = build_bass()
    return _CACHE["nc"]


def make_in_maps(x, W_K, W_Q, W_V):
    x = np.asarray(x, dtype=np.float32)
    in_maps = []
    for c in range(NCORES):
        b = c // 2
        hb = (c % 2) * HPB
        xT = np.ascontiguousarray(x[b].T).astype(ml_dtypes.bfloat16)

        def stat(w):  # stationary layout for Q/K: [P, PAIRS, KD, H2]
            w = np.asarray(w, dtype=np.float32)
            arr = np.empty((P, PAIRS, KD, H2), np.float32)
            for p in range(PAIRS):
                wp = w[hb + 2 * p : hb + 2 * p + 2].reshape(H2, D).T  # [D, H2]
                arr[:, p, :, :] = wp.reshape(KD, P, H2).transpose(1, 0, 2)
            return np.ascontiguousarray(
                arr.reshape(P, PAIRS * KD * H2)
            ).astype(ml_dtypes.bfloat16)

        def mov(w):  # moving layout for V: [P, KD, HPB*DH]
            w = np.asarray(w, dtype=np.float32)
            wt = w[hb : hb + HPB].reshape(HPB * DH, D).T  # [D, 8*64] head-major
            wt = wt.reshape(KD, P, HPB * DH).transpose(1, 0, 2)
            return np.ascontiguousarray(
                wt.reshape(P, KD * HPB * DH)
            ).astype(ml_dtypes.bfloat16)

        in_maps.append(
            {"xT": xT, "wq": stat(W_Q), "wk": stat(W_K), "wv": mov(W_V)}
        )
    return in_maps


def kernel(x, W_K, W_Q, W_V, _trace=False, _trace_kwargs=None):
    in_maps = make_in_maps(x, W_K, W_Q, W_V)
    res = run_bass_kernel_spmd(
        _get_nc(),
        in_maps,
        list(range(NCORES)),
        trace=_trace,
        **(_trace_kwargs or {}),
    )
    _CACHE["last_results"] = res
    out = np.empty((B, T, NH * DH), np.float32)
    for c in range(NCORES):
        zt = np.asarray(res.results[c]["out"])  # [PAIRS, 2, DH+1, T]
        z = zt[:, :, :DH, :] / zt[:, :, DH : DH + 1, :]
        b = c // 2
        hb = (c % 2) * HPB
        for p in range(PAIRS):
            for a in range(2):
                h = hb + 2 * p + a
                out[b, :, h * DH : (h + 1) * DH] = z[p, a].T
    return out


# revision 13
# speedup vs baseline: 1.1046x; 1.0217x over previous
"""Causal multi-head attention (B=4, T=2048, D=1024, 16 heads x 64) on 8 trn2 cores.

Sharding: batch x heads hybrid. Core c owns batch c//2 and heads
(c%2)*8 .. +8 (four packed head-pairs of 128). Each core receives x^T for its
batch ([D, T] bf16) plus its 8 heads' W_Q/W_K (stationary layout) and W_V^T
(moving layout); it computes full causal attention for its heads and writes
z^T plus the softmax denominator row per head ([PAIRS, 2, 65, T] f32). The
host normalizes and re-lays-out.

Device kernel design (per core):
  - Q^T/K^T projections as 8-matmul accumulation units per (pair, 512-col
    quarter); stationary weights, moving x (N=512, full PE rate).
  - V produced directly in [t, h] orientation: stationary x^T[d,t] chunk,
    moving W_V^T (N=512) -> no PE transposes at all; DVE copies slice the
    PSUM result into per-(pair,head) v_aug tiles ([128 kt, 16 chunk, 64 v |
    ones | pad]) whose ones-column yields the softmax denominator for free.
  - scores computed transposed S^T[kt, qt]; the two heads of a pair run as
    concurrent PE row-tiles (K=64 at partition bases 0/64) into the two
    halves of one [128, 1024] f32 PSUM; ONE exp (scale=1/8 fused) per chunk
    covers both heads (diagonal chunks span the dead middle columns; that
    garbage is never read downstream).
  - causal mask applied post-exp with gpsimd affine_select (idle engine).
  - AV is software-pipelined a few chunks behind the scores and accumulates
    z^T + denominator in f32 PSUM.
  - THE SCHEDULING POINT: exp on ScalarE (~166us/core) and matmuls on PE
    (~169us/core) are nearly balanced, and engine queues are FIFO - so the
    emission order interleaves projection/V units between attention chunks
    (ratio-paced + deadline-forced) across ALL four pairs' attention,
    keeping both engines busy instead of serializing phase by phase.
"""

import os
import sys

for _p in ("/opt/trn_rl_repo", "/root/.axon_site/_ro/trn_rl_repo"):
    if os.path.isdir(_p) and _p not in sys.path:
        sys.path.insert(0, _p)

import ml_dtypes
import numpy as np

import concourse.bass as bass
import concourse.mybir as mybir
import concourse.tile as tile
from concourse import bacc
from concourse.bass import ds
from concourse.bass_utils import run_bass_kernel_spmd

B, T, D = 4, 2048, 1024
NH, DH = 16, 64
NCORES = 8
HPB = 8                     # heads per core (batch x head sharding)
PAIRS = HPB // 2            # packed head-pairs per core = 4
H2 = 2 * DH                 # packed pair dim = 128
P = 128
QT = 512                    # query-tile width (psum bank limit for f32 out)
NQ4 = T // QT               # 4 query tiles
NCH = T // P                # 16 key chunks
KD = D // P                 # 8 contraction chunks
F32 = mybir.dt.float32
BF16 = mybir.dt.bfloat16
SCALE = 1.0 / np.sqrt(DH)   # 0.125


def _build(nc, tc, xT_d, w_d, out_d):
    from contextlib import ExitStack

    AF = mybir.ActivationFunctionType
    OP = mybir.AluOpType

    with ExitStack() as ctx:
        ep = ctx.enter_context
        const = ep(tc.tile_pool(name="const", bufs=1))
        xt_pool = ep(tc.tile_pool(name="xt", bufs=1))
        qk_pool = ep(tc.tile_pool(name="qk", bufs=1))
        va_pool = ep(tc.tile_pool(name="va", bufs=1))
        p_pool = ep(tc.tile_pool(name="pp", bufs=8))
        zt_pool = ep(tc.tile_pool(name="zt", bufs=4))
        ps_acc = ep(tc.tile_pool(name="ps_acc", bufs=2, space="PSUM"))
        ps_s = ep(tc.tile_pool(name="ps_s", bufs=2, space="PSUM"))
        ps_z = ep(tc.tile_pool(name="ps_z", bufs=2, space="PSUM"))

        # ---- DMA order matters: pair-0 Q/K weights, then the first x^T
        # quarter, so the first projection unit can start ~1.5us in ----
        w_sb = {}
        for name in ("wq", "wk"):
            w_sb[name] = const.tile(
                [P, PAIRS, KD, H2], BF16, tag=name, name=name
            )
        wv_sb = const.tile([P, KD, HPB * DH], BF16, tag="wv", name="wv_sb")
        xt = [
            xt_pool.tile([P, T], BF16, tag=f"x{k}", name="xt") for k in range(KD)
        ]

        def dma_w_pair(p):
            for name in ("wq", "wk"):
                nc.sync.dma_start(
                    w_sb[name][:, p],
                    w_d[name].rearrange(
                        "p (r c h) -> p r c h", r=PAIRS, c=KD
                    )[:, p],
                )

        dma_w_pair(0)
        for k in range(KD):
            nc.sync.dma_start(
                xt[k][:, ds(0, QT)], xT_d[ds(k * P, P), ds(0, QT)]
            )
        nc.sync.dma_start(wv_sb[:], w_d["wv"].rearrange("p (c h) -> p c h", c=KD))
        dma_w_pair(1)
        for t4 in range(1, NQ4):
            for k in range(KD):
                nc.sync.dma_start(
                    xt[k][:, ds(t4 * QT, QT)], xT_d[ds(k * P, P), ds(t4 * QT, QT)]
                )
            if t4 + 1 < PAIRS:
                dma_w_pair(t4 + 1)

        qt = [
            qk_pool.tile([P, T], BF16, tag=f"qt{p}", name="qt") for p in range(PAIRS)
        ]
        kt = [
            qk_pool.tile([P, T], BF16, tag=f"kt{p}", name="kt") for p in range(PAIRS)
        ]
        # v_aug: [kt, chunk, 64 v-cols | ones col | 63 zeros]
        va = [
            [
                va_pool.tile([P, NCH, P], BF16, tag=f"v{p}{a}", name="va")
                for a in range(2)
            ]
            for p in range(PAIRS)
        ]
        for p in range(PAIRS):
            for a in range(2):
                nc.gpsimd.memset(va[p][a][:, :, DH:P], 0.0)
                nc.gpsimd.memset(va[p][a][:, :, DH : DH + 1], 1.0)

        # ---- background work units (projections + V), emitted between
        # attention chunks by the driver below ----
        def qk_unit(name, p, t4):
            dst = qt[p] if name == "wq" else kt[p]

            def emit():
                acc = ps_acc.tile([P, QT], F32, tag="acc", name="acc")
                for k in range(KD):
                    nc.tensor.matmul(
                        acc[:],
                        w_sb[name][:, p, k, :],
                        xt[k][:, ds(t4 * QT, QT)],
                        start=(k == 0),
                        stop=(k == KD - 1),
                    )
                nc.vector.tensor_copy(dst[:, ds(t4 * QT, QT)], acc[:])

            return emit

        def v_unit(j):
            def emit():
                pv = ps_acc.tile([P, HPB * DH], F32, tag="acc", name="pv")
                for k in range(KD):
                    nc.tensor.matmul(
                        pv[:],
                        xt[k][:, ds(j * P, P)],
                        wv_sb[:, k, :],
                        start=(k == 0),
                        stop=(k == KD - 1),
                    )
                for p in range(PAIRS):
                    for a in range(2):
                        nc.vector.tensor_copy(
                            va[p][a][:, j, 0:DH], pv[:, ds((2 * p + a) * DH, DH)]
                        )

            return emit

        # ---- EDF schedule of projection/V units between attention chunks.
        # Tiles run q4-major, pair-minor; each unit gets a chunk-indexed
        # deadline (the chunk before which it must be emitted), and a rate
        # pacer walks the deadline-sorted list early so the PE never takes
        # a multi-unit burst that starves the exp stream. ----
        MPB = QT // P
        tile_start = {}
        cs = 0
        for q4 in range(NQ4):
            for p in range(PAIRS):
                tile_start[(q4, p)] = cs
                cs += (q4 + 1) * MPB
        total_chunks = cs  # 160

        LAG = 3  # chunks the AV matmuls trail the score/exp stream by
        units = []  # (deadline, order, emit_fn)
        for p in range(PAIRS):
            for t4 in range(NQ4):
                # qt quarter q4 is read by every chunk of tile (q4, p);
                # kt quarter t4 is first read at local chunk 4*t4
                units.append((tile_start[(t4, p)], 0, qk_unit("wq", p, t4)))
                units.append(
                    (tile_start[(t4, p)] + t4 * MPB, 1, qk_unit("wk", p, t4))
                )
        for j in range(NCH):
            q4 = j // MPB
            # v chunk j is first read by the trailing AV of tile (q4, 0)
            dl = tile_start[(q4, 0)] + (j - q4 * MPB) + LAG
            units.append((dl, 2, v_unit(j)))
        units.sort(key=lambda u: (u[0], u[1]))
        n_units = len(units)
        uidx = [0]

        def inject_bg(c):
            # deadline-forced, then rate-paced (n_units spread over chunks)
            while uidx[0] < n_units and units[uidx[0]][0] <= c:
                units[uidx[0]][2]()
                uidx[0] += 1
            while (uidx[0] < n_units
                   and uidx[0] * total_chunks < n_units * c):
                units[uidx[0]][2]()
                uidx[0] += 1

        # ---- attention: one global chunk stream (q4-major, pair-minor);
        # the AV matmuls trail the score/exp stream by LAG chunks even
        # across tile boundaries, so the next tile's scores keep the exp
        # pipeline fed while the previous tile's AV+drain runs ----
        zrow = DH + 1
        stream = []
        for q4 in range(NQ4):
            for p in range(PAIRS):
                njs = (q4 + 1) * MPB
                for j in range(njs):
                    stream.append((q4, p, j, njs))

        pend = []  # (q4, p, j, njs, c0, pe) awaiting the AV matmuls
        pz_map = {}

        def emit_av():
            q4, p, jj, njs, cc0, ppe = pend.pop(0)
            if jj == 0:
                pz_map[(q4, p)] = [
                    ps_z.tile([P, QT], F32, tag="z", name="pz") for _ in range(2)
                ]
            pz = pz_map[(q4, p)]
            for a in range(2):
                nc.tensor.matmul(
                    pz[a][:, cc0:QT],
                    va[p][a][:, jj, :],
                    ppe[:, a * QT + cc0 : (a + 1) * QT],
                    start=(jj == 0),
                    stop=(jj == njs - 1),
                    skip_group_check=True,
                )
            if jj == njs - 1:
                for a in range(2):
                    zt_t = zt_pool.tile([zrow, QT], F32, tag="zt", name="zt_t")
                    nc.vector.tensor_copy(zt_t[:], pz[a][0:zrow, :])
                    nc.sync.dma_start(out_d[p, a, :, ds(q4 * QT, QT)], zt_t[:])
                del pz_map[(q4, p)]

        for c, (q4, p, j, njs) in enumerate(stream):
            inject_bg(c)
            rdiag = j - q4 * MPB
            c0 = 0 if rdiag < 0 else rdiag * P
            w_hi = (rdiag + 1) * P if rdiag >= 0 else 0
            nw = QT - c0
            ss = ps_s.tile([P, 2 * QT], F32, tag="s", name="ss")
            pe = p_pool.tile([P, 2 * QT], BF16, tag="p", name="pe")
            for a in range(2):
                hp = ds(a * DH, DH)
                nc.tensor.matmul(
                    ss[:, a * QT + c0 : (a + 1) * QT],
                    kt[p][hp, ds(j * P, P)],
                    qt[p][hp, ds(q4 * QT + c0, nw)],
                    start=True,
                    stop=True,
                )
            # one exp covering both heads' valid column blocks via a
            # strided AP (no read of the dead middle columns)
            nc.scalar.activation(
                pe.rearrange("p (a q) -> p a q", a=2)[:, :, c0:QT],
                ss.rearrange("p (a q) -> p a q", a=2)[:, :, c0:QT],
                AF.Exp,
                scale=float(SCALE),
            )
            if rdiag >= 0:
                # keep iff qt >= kt  <=>  (col - p - 128*rdiag) >= 0
                for a in range(2):
                    nc.gpsimd.affine_select(
                        out=pe[:, a * QT + c0 : a * QT + w_hi],
                        in_=pe[:, a * QT + c0 : a * QT + w_hi],
                        compare_op=OP.is_ge,
                        fill=0.0,
                        base=c0 - rdiag * P,
                        pattern=[[1, w_hi - c0]],
                        channel_multiplier=-1,
                    )
            pend.append((q4, p, j, njs, c0, pe))
            if len(pend) > LAG:
                emit_av()
        while pend:
            emit_av()

        # drain any remaining units (shouldn't happen)
        while uidx[0] < n_units:
            units[uidx[0]][2]()
            uidx[0] += 1


def build_bass():
    nc = bacc.Bacc(None, target_bir_lowering=False)
    xT_d = nc.declare_dram_parameter("xT", [D, T], BF16, isOutput=False)
    w_d = {
        "wq": nc.declare_dram_parameter(
            "wq", [P, KD * PAIRS * H2], BF16, isOutput=False
        ),
        "wk": nc.declare_dram_parameter(
            "wk", [P, KD * PAIRS * H2], BF16, isOutput=False
        ),
        "wv": nc.declare_dram_parameter(
            "wv", [P, KD * HPB * DH], BF16, isOutput=False
        ),
    }
    out_d = nc.declare_dram_parameter(
        "out", [PAIRS, 2, DH + 1, T], F32, isOutput=True
    )
    with tile.TileContext(nc) as tc:
        _build(nc, tc, xT_d, w_d, out_d)
    nc.compile()
    return nc


_CACHE = {}


def _get_nc():
    if "nc" not in _CACHE:
        _CACHE["nc"] = build_bass()
    return _CACHE["nc"]


def make_in_maps(x, W_K, W_Q, W_V):
    x = np.asarray(x, dtype=np.float32)
    in_maps = []
    for c in range(NCORES):
        b = c // 2
        hb = (c % 2) * HPB
        xT = np.ascontiguousarray(x[b].T).astype(ml_dtypes.bfloat16)

        def stat(w):  # stationary layout for Q/K: [P, PAIRS, KD, H2]
            w = np.asarray(w, dtype=np.float32)
            arr = np.empty((P, PAIRS, KD, H2), np.float32)
            for p in range(PAIRS):
                wp = w[hb + 2 * p : hb + 2 * p + 2].reshape(H2, D).T  # [D, H2]
                arr[:, p, :, :] = wp.reshape(KD, P, H2).transpose(1, 0, 2)
            return np.ascontiguousarray(
                arr.reshape(P, PAIRS * KD * H2)
            ).astype(ml_dtypes.bfloat16)

        def mov(w):  # moving layout for V: [P, KD, HPB*DH]
            w = np.asarray(w, dtype=np.float32)
            wt = w[hb : hb + HPB].reshape(HPB * DH, D).T  # [D, 8*64] head-major
            wt = wt.reshape(KD, P, HPB * DH).transpose(1, 0, 2)
            return np.ascontiguousarray(
                wt.reshape(P, KD * HPB * DH)
            ).astype(ml_dtypes.bfloat16)

        in_maps.append(
            {"xT": xT, "wq": stat(W_Q), "wk": stat(W_K), "wv": mov(W_V)}
        )
    return in_maps


def kernel(x, W_K, W_Q, W_V, _trace=False, _trace_kwargs=None):
    in_maps = make_in_maps(x, W_K, W_Q, W_V)
    res = run_bass_kernel_spmd(
        _get_nc(),
        in_maps,
        list(range(NCORES)),
        trace=_trace,
        **(_trace_kwargs or {}),
    )
    _CACHE["last_results"] = res
    out = np.empty((B, T, NH * DH), np.float32)
    for c in range(NCORES):
        zt = np.asarray(res.results[c]["out"])  # [PAIRS, 2, DH+1, T]
        z = zt[:, :, :DH, :] / zt[:, :, DH : DH + 1, :]
        b = c // 2
        hb = (c % 2) * HPB
        for p in range(PAIRS):
            for a in range(2):
                h = hb + 2 * p + a
                out[b, :, h * DH : (h + 1) * DH] = z[p, a].T
    return out
